# revision 3
# baseline (speedup 1.0000x reference)
"""Edge-decoder (GNN link prediction) kernel for 8 Trainium2 NeuronCores.

Computes logits[e] = sum_d x[src[e], d] * x[tar[e], d] for 640K edges
(pos then neg), node table x [100000, 128] f32.

Strategy: edges bucketed by (src_node // 25000, tar_node // 25000) -> 16
buckets so node offsets fit int16 against a 25000-row chunk of x; each
bucket's rows fetched with one dma_gather per side, two buckets in flight
across the 4 SWDGE queues. HW time tracks the STATIC idx slot count
(~8.5ns/slot/queue-pair, independent of runtime counts), so:
- Balanced bucketing: each global bucket's edges are dealt round-robin
  across cores -> per-(core,bucket) counts ~equal, cap 5120 vs 5248, and
  all cores/queues carry identical slot loads.
- All 32 idx tiles are preloaded with ONE dma_start into a single SBUF
  tile (no mid-stream idx DMA dependencies or dispatches).
- Logits accumulate in one SBUF tile and are stored once at the end
  (1 output DMA instead of 16 small ones).
DVE does a tensor_tensor multiply (bf16 product) + tensor_reduce per
bucket, fully overlapped with the gathers of the next bucket pair.
"""

import numpy as np

N_NODES = 100000
D = 128
E_TOTAL = 640000
N_CORES = 8
N_CHUNK = 4
CHUNK = N_NODES // N_CHUNK  # 25000 < 32768 so offsets fit int16
NB = N_CHUNK * N_CHUNK  # 16 buckets

_cached = {}


def build(cap, dyn_repeat=False, unroll=1):
    from concourse import bacc, mybir, tile

    G = cap // 128
    W = cap // 16
    nc = bacc.Bacc(
        "TRN2",
        target_bir_lowering=False,
        debug=False,
        num_devices=N_CORES,
        num_swdge_queues=4,
    )
    x = nc.dram_tensor(
        "x", [N_NODES, D], mybir.dt.float32, kind="ExternalInput"
    ).ap()
    idx_all = nc.dram_tensor(
        "idx_all", [128, 2, NB, W], mybir.dt.int16, kind="ExternalInput"
    ).ap()
    counts = nc.dram_tensor(
        "counts", [1, NB], mybir.dt.int32, kind="ExternalInput"
    ).ap()
    if dyn_repeat:
        reps = nc.dram_tensor(
            "reps", [1, 1], mybir.dt.int32, kind="ExternalInput"
        ).ap()
    logits = nc.dram_tensor(
        "logits", [128, NB * G], mybir.dt.float32, kind="ExternalOutput"
    ).ap()

    with tile.TileContext(nc) as tc:
        with tc.tile_pool(name="misc", bufs=1) as miscp, tc.tile_pool(
            name="gat", bufs=2
        ) as gatp, tc.tile_pool(name="prod", bufs=1) as prodp:
            cnt_t = miscp.tile([1, NB], mybir.dt.int32)
            nc.sync.dma_start(out=cnt_t[:], in_=counts)
            if dyn_repeat:
                reps_t = miscp.tile([1, 1], mybir.dt.int32)
                nc.sync.dma_start(out=reps_t[:], in_=reps)
            idx_t = miscp.tile([128, 2, NB, W], mybir.dt.int16)
            lg_all = miscp.tile([128, NB * G], mybir.dt.float32)

            def body():
                def bucket_pair(b0):
                    tiles = {}
                    for j, b in enumerate((b0, b0 + 1)):
                        bs, bt = b // N_CHUNK, b % N_CHUNK
                        n_reg = nc.values_load(
                            cnt_t[0:1, b : b + 1],
                            engines=(mybir.EngineType.Pool,),
                            min_val=1,
                            max_val=cap,
                            skip_runtime_bounds_check=True,
                        )
                        s_t = gatp.tile(
                            [128, G, D], mybir.dt.float32, tag=f"s{j}"
                        )
                        nc.gpsimd.dma_gather(
                            s_t[:],
                            x[bs * CHUNK : (bs + 1) * CHUNK, :],
                            idx_t[:, 0, b, :],
                            cap,
                            n_reg,
                            D,
                            single_packet=False,
                            queue_num=2 * j,
                        )
                        t_t = gatp.tile(
                            [128, G, D], mybir.dt.float32, tag=f"t{j}"
                        )
                        nc.gpsimd.dma_gather(
                            t_t[:],
                            x[bt * CHUNK : (bt + 1) * CHUNK, :],
                            idx_t[:, 1, b, :],
                            cap,
                            n_reg,
                            D,
                            single_packet=False,
                            queue_num=2 * j + 1,
                        )
                        tiles[b] = (s_t, t_t)
                    for j, b in enumerate((b0, b0 + 1)):
                        s_t, t_t = tiles[b]
                        p_t = prodp.tile(
                            [128, G, D], mybir.dt.bfloat16, tag="p"
                        )
                        nc.vector.tensor_tensor(
                            out=p_t[:],
                            in0=s_t[:],
                            in1=t_t[:],
                            op=mybir.AluOpType.mult,
                        )
                        nc.vector.tensor_reduce(
                            out=lg_all[:, b * G : (b + 1) * G].unsqueeze(-1),
                            in_=p_t[:],
                            axis=mybir.AxisListType.X,
                            op=mybir.AluOpType.add,
                        )

                for b0 in range(0, NB, 2):
                    bucket_pair(b0)
                nc.scalar.dma_start(out=logits, in_=lg_all[:])

            nc.scalar.dma_start(out=idx_t[:], in_=idx_all)
            if dyn_repeat:
                r_val = nc.values_load(
                    reps_t[0:1, 0:1],
                    min_val=1,
                    max_val=1000,
                    skip_runtime_bounds_check=True,
                )
                with tc.For_i(0, r_val):
                    for _ in range(unroll):
                        body()
            else:
                body()
    nc.compile()
    return nc


def _get_nc(cap, dyn_repeat=False, unroll=1):
    key = (cap, dyn_repeat, unroll)
    if key not in _cached:
        _cached[key] = build(cap, dyn_repeat, unroll)
    return _cached[key]


def host_prepare(x, src, tar, cap=None):
    """Balanced bucketing: each global (src-chunk, tar-chunk) bucket's edges
    are dealt round-robin across the 8 cores. Returns (in_maps, restore, cap);
    restore[c] = (ids_per_bucket, cnts) for reassembly."""
    x_f = np.ascontiguousarray(np.asarray(x, np.float32))
    b_all = (src // CHUNK) * N_CHUNK + (tar // CHUNK)
    order_all = np.argsort(b_all, kind="stable")
    cnts_all = np.bincount(b_all, minlength=NB)

    per_core_ids = [[] for _ in range(N_CORES)]
    per_core_cnts = np.zeros((N_CORES, NB), np.int32)
    pos = 0
    for bi in range(NB):
        n = int(cnts_all[bi])
        ids = order_all[pos : pos + n]
        pos += n
        for c in range(N_CORES):
            sel = ids[c::N_CORES]
            per_core_ids[c].append(sel)
            per_core_cnts[c, bi] = len(sel)

    max_n = int(per_core_cnts.max())
    if cap is None:
        cap = ((max_n + 127) // 128) * 128
    assert max_n <= cap

    in_maps, restore = [], []
    for c in range(N_CORES):
        sflat = np.full((NB, cap), -1, np.int16)
        tflat = np.full((NB, cap), -1, np.int16)
        counts_eff = per_core_cnts[c].copy()
        for bi in range(NB):
            ids = per_core_ids[c][bi]
            n = len(ids)
            if n == 0:  # Q7/ucode need >= 1 valid index
                sflat[bi, 0] = 0
                tflat[bi, 0] = 0
                counts_eff[bi] = 1
                continue
            s = src[ids]
            t = tar[ids]
            sflat[bi, :n] = (s - (s // CHUNK) * CHUNK).astype(np.int16)
            tflat[bi, :n] = (t - (t // CHUNK) * CHUNK).astype(np.int16)

        def wrap(flat):
            # logical idx i -> partition i%16, col i//16; replicate 8x
            return np.tile(
                flat.reshape(NB, cap // 16, 16).transpose(0, 2, 1), (1, 8, 1)
            )

        ia = np.stack([wrap(sflat), wrap(tflat)])  # [2, NB, 128, W]
        in_maps.append(
            {
                "x": x_f,
                "idx_all": np.ascontiguousarray(ia.transpose(2, 0, 1, 3)),
                "counts": counts_eff[None, :],
            }
        )
        restore.append((per_core_ids[c], per_core_cnts[c]))
    return in_maps, restore, cap


def assemble(results, restore):
    out = np.empty((E_TOTAL, 1), np.float32)
    for c in range(N_CORES):
        lg = np.asarray(results[c]["logits"]).reshape(128, NB, -1)
        ids_per_bucket, cnts = restore[c]
        flat = lg.transpose(1, 2, 0).reshape(NB, -1)  # [NB, G*128]
        for bi in range(NB):
            ids = ids_per_bucket[bi]
            if len(ids):
                out[ids, 0] = flat[bi, : len(ids)]
    return out


def kernel(x, pos_edge_index, neg_edge_index):
    from concourse.bass_utils import run_bass_kernel_spmd

    src = np.concatenate(
        [np.asarray(pos_edge_index[0]), np.asarray(neg_edge_index[0])]
    ).astype(np.int32)
    tar = np.concatenate(
        [np.asarray(pos_edge_index[1]), np.asarray(neg_edge_index[1])]
    ).astype(np.int32)

    in_maps, restore, cap = host_prepare(np.asarray(x), src, tar)
    nc = _get_nc(cap)
    res = run_bass_kernel_spmd(nc, in_maps, core_ids=list(range(N_CORES)))
    return assemble(res.results, restore)


# revision 4
# speedup vs baseline: 1.2714x; 1.2714x over previous
"""Edge-decoder (GNN link prediction) kernel for 8 Trainium2 NeuronCores.

Computes logits[e] = sum_d x[src[e], d] * x[tar[e], d] for 640K edges,
node table x [100000, 128] f32. Edges bucketed by (src//25000, tar//25000)
-> 16 buckets (int16 offsets vs a 25000-row chunk); one dma_gather per
bucket-side, two buckets in flight across the 4 SWDGE queues; DVE multiply
(bf16 product) + reduce overlapped with the next pair's gathers. HW time
tracks the STATIC idx slot count (~8.5ns/slot/queue-pair, independent of
runtime counts), hence:
- Balanced bucketing: each global bucket's edges dealt round-robin across
  cores (per-core counts differ by <=1) -> minimal shared static widths.
- Per-bucket exact static num_idxs = roundup16(max per-core count), idx
  arrays padded with VALID zeros (garbage dropped at assembly) -> the
  count is a compile-time constant: no counts input, no values_load.
- All idx tiles preloaded with ONE dma_start; logits accumulate in one
  SBUF tile, stored once at the end.

Each bucket b gets a compile-time num_idxs_b = roundup16(max_core count_b)
(the balanced dealing makes per-core counts differ by <=1, so the shared
width wastes <=15 slots). Idx arrays are padded to num_idxs_b with VALID
zeros (gathers row 0, dropped at assembly), so num_idxs_reg is the python
int num_idxs_b — no counts input, no values_load, no -1 trimming.
"""

import numpy as np

N_NODES = 100000
D = 128
E_TOTAL = 640000
N_CORES = 8
N_CHUNK = 4
CHUNK = N_NODES // N_CHUNK
NB = N_CHUNK * N_CHUNK

_cached = {}


def build(widths, dyn_repeat=False, unroll=1):
    """widths: tuple of NB per-bucket num_idxs (multiples of 16)."""
    from concourse import bacc, mybir, tile

    gbs = [(w + 127) // 128 for w in widths]  # out-tile groups per bucket
    Gmax = max(gbs)
    woff = np.concatenate([[0], np.cumsum([w // 16 for w in widths])])
    goff = np.concatenate([[0], np.cumsum(gbs)])
    Wtot = int(woff[-1])  # total idx cols (16 idx each)
    Gtot = int(goff[-1])

    nc = bacc.Bacc(
        "TRN2",
        target_bir_lowering=False,
        debug=False,
        num_devices=N_CORES,
        num_swdge_queues=4,
    )
    x = nc.dram_tensor(
        "x", [N_NODES, D], mybir.dt.float32, kind="ExternalInput"
    ).ap()
    idx_all = nc.dram_tensor(
        "idx_all", [128, 2 * Wtot], mybir.dt.int16, kind="ExternalInput"
    ).ap()
    if dyn_repeat:
        reps = nc.dram_tensor(
            "reps", [1, 1], mybir.dt.int32, kind="ExternalInput"
        ).ap()
    logits = nc.dram_tensor(
        "logits", [128, Gtot], mybir.dt.float32, kind="ExternalOutput"
    ).ap()

    with tile.TileContext(nc) as tc:
        with tile_pools(tc) as (miscp, gatp, prodp):
            if dyn_repeat:
                reps_t = miscp.tile([1, 1], mybir.dt.int32)
                nc.sync.dma_start(out=reps_t[:], in_=reps)
            idx_t = miscp.tile([128, 2 * Wtot], mybir.dt.int16)
            lg_all = miscp.tile([128, Gtot], mybir.dt.float32)

            def body():
                def bucket_pair(b0):
                    tiles = {}
                    for j, b in enumerate((b0, b0 + 1)):
                        bs, bt = b // N_CHUNK, b % N_CHUNK
                        w = widths[b]
                        gb = gbs[b]
                        s_t = gatp.tile(
                            [128, Gmax, D], mybir.dt.float32, tag=f"s{j}"
                        )
                        nc.gpsimd.dma_gather(
                            s_t[:, :gb, :],
                            x[bs * CHUNK : (bs + 1) * CHUNK, :],
                            idx_t[:, int(woff[b]) : int(woff[b + 1])],
                            w,
                            w,
                            D,
                            single_packet=False,
                            queue_num=2 * j,
                        )
                        t_t = gatp.tile(
                            [128, Gmax, D], mybir.dt.float32, tag=f"t{j}"
                        )
                        nc.gpsimd.dma_gather(
                            t_t[:, :gb, :],
                            x[bt * CHUNK : (bt + 1) * CHUNK, :],
                            idx_t[:, Wtot + int(woff[b]) : Wtot + int(woff[b + 1])],
                            w,
                            w,
                            D,
                            single_packet=False,
                            queue_num=2 * j + 1,
                        )
                        tiles[b] = (s_t, t_t)
                    for j, b in enumerate((b0, b0 + 1)):
                        s_t, t_t = tiles[b]
                        gb = gbs[b]
                        p_t = prodp.tile(
                            [128, Gmax, D], mybir.dt.bfloat16, tag="p"
                        )
                        nc.vector.tensor_tensor(
                            out=p_t[:, :gb, :],
                            in0=s_t[:, :gb, :],
                            in1=t_t[:, :gb, :],
                            op=mybir.AluOpType.mult,
                        )
                        nc.vector.tensor_reduce(
                            out=lg_all[
                                :, int(goff[b]) : int(goff[b + 1])
                            ].unsqueeze(-1),
                            in_=p_t[:, :gb, :],
                            axis=mybir.AxisListType.X,
                            op=mybir.AluOpType.add,
                        )

                for b0 in range(0, NB, 2):
                    bucket_pair(b0)
                nc.scalar.dma_start(out=logits, in_=lg_all[:])

            nc.scalar.dma_start(out=idx_t[:], in_=idx_all)
            if dyn_repeat:
                r_val = nc.values_load(
                    reps_t[0:1, 0:1],
                    min_val=1,
                    max_val=1000,
                    skip_runtime_bounds_check=True,
                )
                with tc.For_i(0, r_val):
                    for _ in range(unroll):
                        body()
            else:
                body()
    nc.compile()
    return nc


def tile_pools(tc):
    import contextlib

    @contextlib.contextmanager
    def pools():
        with tc.tile_pool(name="misc", bufs=1) as miscp, tc.tile_pool(
            name="gat", bufs=2
        ) as gatp, tc.tile_pool(name="prod", bufs=1) as prodp:
            yield miscp, gatp, prodp

    return pools()


def _get_nc(widths, dyn_repeat=False, unroll=1):
    key = (tuple(widths), dyn_repeat, unroll)
    if key not in _cached:
        _cached[key] = build(tuple(widths), dyn_repeat, unroll)
    return _cached[key]


def host_prepare(x, src, tar):
    """Balanced dealing + per-bucket exact widths (roundup16 of the max
    per-core count, valid-0 padded). Returns (in_maps, restore, widths)."""
    x_f = np.ascontiguousarray(np.asarray(x, np.float32))
    b_all = (src // CHUNK) * N_CHUNK + (tar // CHUNK)
    order_all = np.argsort(b_all, kind="stable")
    cnts_all = np.bincount(b_all, minlength=NB)

    per_core_ids = [[] for _ in range(N_CORES)]
    pos = 0
    widths = []
    for bi in range(NB):
        n = int(cnts_all[bi])
        ids = order_all[pos : pos + n]
        pos += n
        for c in range(N_CORES):
            per_core_ids[c].append(ids[c::N_CORES])
        mx = max(1, -(-n // N_CORES))
        widths.append(((mx + 15) // 16) * 16)
    woff = np.concatenate([[0], np.cumsum([w // 16 for w in widths])])
    Wtot = int(woff[-1])

    in_maps, restore = [], []
    for c in range(N_CORES):
        ia = np.zeros((128, 2 * Wtot), np.int16)
        for bi in range(NB):
            ids = per_core_ids[c][bi]
            w = widths[bi]
            sfl = np.zeros(w, np.int16)
            tfl = np.zeros(w, np.int16)
            n = len(ids)
            if n:
                s = src[ids]
                t = tar[ids]
                sfl[:n] = (s - (s // CHUNK) * CHUNK).astype(np.int16)
                tfl[:n] = (t - (t // CHUNK) * CHUNK).astype(np.int16)

            def wrap(flat):
                # logical idx i -> partition i%16, col i//16; replicate 8x
                return np.tile(flat.reshape(-1, 16).T, (8, 1))

            ia[:, int(woff[bi]) : int(woff[bi + 1])] = wrap(sfl)
            ia[:, Wtot + int(woff[bi]) : Wtot + int(woff[bi + 1])] = wrap(tfl)
        in_maps.append({"x": x_f, "idx_all": np.ascontiguousarray(ia)})
        restore.append((per_core_ids[c], widths))
    return in_maps, restore, widths


def assemble(results, restore):
    widths = restore[0][1]
    gbs = [(w + 127) // 128 for w in widths]
    goff = np.concatenate([[0], np.cumsum(gbs)])
    out = np.empty((E_TOTAL, 1), np.float32)
    for c in range(N_CORES):
        lg = np.asarray(results[c]["logits"])  # [128, Gtot]
        for bi in range(NB):
            ids = restore[c][0][bi]
            if len(ids):
                blk = lg[:, int(goff[bi]) : int(goff[bi + 1])]
                flat = blk.T.reshape(-1)  # slot i at flat[i//128*128... ]
                # slot i -> partition i%128, group i//128: flat order is
                # (group, partition) so slot i == flat[(i//128)*128 + i%128]
                out[ids, 0] = flat[: len(ids)]
    return out


def kernel(x, pos_edge_index, neg_edge_index):
    from concourse.bass_utils import run_bass_kernel_spmd

    src = np.concatenate(
        [np.asarray(pos_edge_index[0]), np.asarray(neg_edge_index[0])]
    ).astype(np.int32)
    tar = np.concatenate(
        [np.asarray(pos_edge_index[1]), np.asarray(neg_edge_index[1])]
    ).astype(np.int32)

    in_maps, restore, widths = host_prepare(np.asarray(x), src, tar)
    nc = _get_nc(widths)
    res = run_bass_kernel_spmd(nc, in_maps, core_ids=list(range(N_CORES)))
    return assemble(res.results, restore)
